# revision 26
# baseline (speedup 1.0000x reference)
"""LSTMCell forward on 8 Trainium2 NeuronCores (Bass/Tile, SPMD data-parallel).

Strategy (transposed, gate-on-partition layout):
  - Shard the batch (32768) across 8 cores: 4096 rows each.
  - Compute z^T = W^T @ xh^T with the GATE dim on PSUM partitions and the
    batch on the free dim. W chunk [128k, 128g] is the stationary operand
    (float32r, self-loading — measured fastest matmul pattern on HW),
    xh^T [128k, 512] the moving one. Benefits vs batch-on-partition:
      * the gate bias becomes a per-partition scalar, so the ACT engine
        fuses bias + sigmoid/tanh into the PSUM evacuation (no 2048-wide
        DVE bias-add);
      * each gate tile's PSUM is [128, 512] f32 = ONE 2 KiB bank, so 8
        accumulation groups are in flight and the PE never waits for
        evacuation.
  - Gate emission order per h-tile j is [g, i, f, o]: ig/fc/cn/tanh(cn)
    complete during o's matmuls, so each j's tail is evac(o) -> hn -> store
    (short loop-seam drain in the timing NEFF).
  - Epilogue per h-tile j: DVE does i*g, f*C, their sum and o*tanh(C_new);
    ACT does the gate activations and tanh(C_new).
  - Block n's output stores are dispatched after block n+1's input loads:
    an out-DMA dispatch waits on its data, and on the in-order SP queue
    that wait would stall the next block's prefetch (measured ~3us/body).
  - Host-side prep: xh^T, C^T per core; outputs C_new^T / h_new^T are
    transposed back on the host. Bias laid out as [128, 16] per-partition
    scalars.
"""
import sys
from contextlib import nullcontext

if "/opt/trn_rl_repo" not in sys.path:
    sys.path.insert(0, "/opt/trn_rl_repo")

import numpy as np
import concourse.bass as bass
import concourse.mybir as mybir
from concourse.tile import TileContext
from concourse.bass_utils import run_bass_kernel_spmd

F32 = mybir.dt.float32
F32R = mybir.dt.float32r
AF = mybir.ActivationFunctionType

N_CORES = 8
P = 128
DH = 512
DH4 = 4 * DH            # 2048
K = 1024                # concat(x, h) contraction dim
KT = K // P             # 8 k-chunks
B_FULL = 32768
B_CORE = B_FULL // N_CORES   # 4096
NB = 512                     # batch columns per block
NBLK = B_CORE // NB          # 8
NJ = DH // P                 # 4 h-dim tiles of 128
NT = 4 * NJ                  # 16 gate tiles of 128


def fanout_multi_waits(nc):
    """This walrus build rejects >1 sync wait per instruction: fan extra
    waits out onto single-wait NoOps on the same (in-order) engine."""
    n = 0
    for f in nc.m.functions:
        for bb in f.blocks:
            new = []
            for inst in bb.instructions:
                si = inst.sync_info
                waits = list(si.on_wait) if si and si.on_wait else []
                if len(waits) > 1:
                    for w in waits[:-1]:
                        nop = mybir.InstNoOp(name=f"waitfan_{n}", ins=[], outs=[])
                        n += 1
                        nop.engine = inst.engine
                        nop.sync_info = mybir.SyncInfo(on_wait=[w], on_update=[])
                        new.append(nop)
                    si.on_wait = [waits[-1]]
                new.append(inst)
            bb.instructions = new
    return n


def build_nc(loop_n=None, out_queue="sp"):
    """Build the per-core program. loop_n wraps the body in a device-side
    For_i repeat (timing probe; outputs unchanged since the body is
    idempotent)."""
    nc = bass.Bass()
    xhT = nc.dram_tensor("xhT", [K, B_CORE], F32R, kind="ExternalInput")
    CT = nc.dram_tensor("CT", [DH, B_CORE], F32, kind="ExternalInput")
    W = nc.dram_tensor("W", [K, DH4], F32R, kind="ExternalInput")
    biasP = nc.dram_tensor("biasP", [P, NT], F32, kind="ExternalInput")
    CnT = nc.dram_tensor("CnT", [DH, B_CORE], F32, kind="ExternalOutput")
    HnT = nc.dram_tensor("HnT", [DH, B_CORE], F32, kind="ExternalOutput")

    xhT_r = xhT[:].rearrange("(kt p) b -> p kt b", p=P)   # [128, 8, 4096]
    W_r = W[:].rearrange("(kt p) g -> p kt g", p=P)       # [128, 8, 2048]
    CT_r = CT[:].rearrange("(j p) b -> p j b", p=P)       # [128, 4, 4096]
    Cn_r = CnT[:].rearrange("(j p) b -> p j b", p=P)
    Hn_r = HnT[:].rearrange("(j p) b -> p j b", p=P)

    with TileContext(nc) as tc:
        with (
            tc.tile_pool(name="const", bufs=1) as const,
            tc.tile_pool(name="io", bufs=2) as io,
            tc.tile_pool(name="zpool", bufs=3) as zpool,
            tc.tile_pool(name="work", bufs=2) as work,
            tc.tile_pool(name="psum", bufs=8, space=bass.MemorySpace.PSUM) as psum,
        ):
            w_t = const.tile([P, KT, DH4], F32R)
            for kt in range(KT):
                nc.sync.dma_start(out=w_t[:, kt, :], in_=W_r[:, kt, :])
            bias_t = const.tile([P, NT], F32)
            nc.sync.dma_start(out=bias_t[:], in_=biasP[:])

            out_eng = {"act": nc.scalar, "pool": nc.gpsimd}.get(out_queue, nc.sync)

            loop = tc.For_i(0, loop_n, 1) if loop_n else nullcontext()
            with loop:
                pending = None
                for blk in range(NBLK):
                    bs = slice(blk * NB, (blk + 1) * NB)
                    last = blk == NBLK - 1
                    # Split the xh load into 4 chunks: after a loop-seam
                    # restart the first matmul waits on 256 KiB, not 1 MiB.
                    xh_t = io.tile([P, KT, NB], F32R, tag="xh")
                    for kc in range(0, KT, 2):
                        nc.sync.dma_start(
                            out=xh_t[:, kc:kc + 2, :], in_=xhT_r[:, kc:kc + 2, bs]
                        )
                    c_t = io.tile([P, NJ, NB], F32, tag="c")
                    nc.sync.dma_start(out=c_t[:], in_=CT_r[:, :, bs])
                    # Block n-1's stores dispatch AFTER block n's loads: an
                    # out-DMA dispatch waits on its data, and on the in-order
                    # SP queue that wait would also stall the next block's
                    # prefetch. By now the data is long ready.
                    if pending is not None:
                        pbs, pcn, phn = pending
                        nc.sync.dma_start(out=Cn_r[:, :, pbs], in_=pcn[:])
                        nc.sync.dma_start(out=Hn_r[:, :, pbs], in_=phn[:])
                        pending = None
                    cn_t = io.tile([P, NJ, NB], F32, tag="cn")
                    hn_t = io.tile([P, NJ, NB], F32, tag="hn")

                    for j in range(NJ):
                        zact = zpool.tile([P, 4, NB], F32, tag="z")
                        # PSUM is evacuated by DVE tensor_copy (full rate on
                        # HW) so the tabled sigmoid/tanh read SBUF — a tabled
                        # activation reading PSUM directly measures ~4x slow.
                        for gi in (3, 0, 1):     # g, i, f
                            t = gi * NJ + j
                            zp = psum.tile([P, NB], F32, tag="zp")
                            for kt in range(KT):
                                nc.tensor.matmul(
                                    zp[:],
                                    w_t[:, kt, t * P:(t + 1) * P],
                                    xh_t[:, kt, :],
                                    start=(kt == 0),
                                    stop=(kt == KT - 1),
                                )
                            zraw = zpool.tile([P, NB], F32, tag="zr")
                            nc.vector.tensor_copy(zraw[:], zp[:])
                            func = AF.Tanh if gi == 3 else AF.Sigmoid
                            nc.scalar.activation(
                                zact[:, gi, :], zraw[:], func,
                                bias=bias_t[:, t:t + 1],
                            )
                        ig = work.tile([P, NB], F32, tag="ig")
                        nc.vector.tensor_mul(ig[:], zact[:, 0, :], zact[:, 3, :])
                        fc = work.tile([P, NB], F32, tag="fc")
                        nc.vector.tensor_mul(fc[:], zact[:, 1, :], c_t[:, j, :])
                        nc.vector.tensor_add(cn_t[:, j, :], fc[:], ig[:])
                        if last:
                            # last block: per-j stores keep the loop-seam
                            # tail down to one 256 KiB transfer
                            out_eng.dma_start(
                                out=Cn_r[:, j, bs], in_=cn_t[:, j, :]
                            )
                        tch = work.tile([P, NB], F32, tag="tch")
                        nc.scalar.activation(tch[:], cn_t[:, j, :], AF.Tanh)
                        t = 2 * NJ + j           # o gate
                        zp = psum.tile([P, NB], F32, tag="zp")
                        for kt in range(KT):
                            nc.tensor.matmul(
                                zp[:],
                                w_t[:, kt, t * P:(t + 1) * P],
                                xh_t[:, kt, :],
                                start=(kt == 0),
                                stop=(kt == KT - 1),
                            )
                        zraw = zpool.tile([P, NB], F32, tag="zr")
                        nc.vector.tensor_copy(zraw[:], zp[:])
                        nc.scalar.activation(
                            zact[:, 2, :], zraw[:], AF.Sigmoid,
                            bias=bias_t[:, t:t + 1],
                        )
                        nc.vector.tensor_mul(hn_t[:, j, :], zact[:, 2, :], tch[:])
                        if last:
                            out_eng.dma_start(
                                out=Hn_r[:, j, bs], in_=hn_t[:, j, :]
                            )
                    if not last:
                        pending = (bs, cn_t, hn_t)
    fanout_multi_waits(nc)
    return nc


_NC = None


def _get_nc():
    global _NC
    if _NC is None:
        _NC = build_nc()
    return _NC


def make_in_maps(x, C, h, Wx, bx, Wh, bh):
    x = np.asarray(x, dtype=np.float32)
    C = np.asarray(C, dtype=np.float32)
    h = np.asarray(h, dtype=np.float32)
    W = np.concatenate(
        [np.asarray(Wx, np.float32), np.asarray(Wh, np.float32)], axis=0
    )                                                       # [1024, 2048]
    bias = np.asarray(bx, np.float32) + np.asarray(bh, np.float32)
    biasP = np.ascontiguousarray(bias.reshape(NT, P).T)     # [128, 16]
    in_maps = []
    for c in range(N_CORES):
        sl = slice(c * B_CORE, (c + 1) * B_CORE)
        xh = np.concatenate([x[sl], h[sl]], axis=1)         # [4096, 1024]
        in_maps.append(
            {
                "xhT": np.ascontiguousarray(xh.T),          # [1024, 4096]
                "CT": np.ascontiguousarray(C[sl].T),        # [512, 4096]
                "W": W,
                "biasP": biasP,
            }
        )
    return in_maps


def kernel(x, C, h, Wx, bx, Wh, bh):
    nc = _get_nc()
    in_maps = make_in_maps(x, C, h, Wx, bx, Wh, bh)
    res = run_bass_kernel_spmd(nc, in_maps, list(range(N_CORES)))
    C_new = np.concatenate(
        [np.ascontiguousarray(res.results[c]["CnT"].T) for c in range(N_CORES)],
        axis=0,
    )
    h_new = np.concatenate(
        [np.ascontiguousarray(res.results[c]["HnT"].T) for c in range(N_CORES)],
        axis=0,
    )
    return (C_new, h_new)
